# revision 54
# baseline (speedup 1.0000x reference)
"""GCN 2-layer kernel for Trainium2 (8 NeuronCores, Bass/Tile), v4.

Pipeline (per core, SPMD across 8 cores):
  - Nodes sharded across cores by degree-balanced snake; within a core,
    round-based LPT assigns nodes to 98 dst blocks of 128 rows.
  - Prep: g1 = (dis*x) @ W1 (dis folded into x on host), written to DRAM
    bounce pieces; partial AllGathers fire per piece as soon as written,
    into a piece-major global bf16 table [piece][core][rows] (contiguous
    collective outputs), overlapped with remaining prep.
  - Edge phase per layer: edges grouped by (dst superblock of 16 blocks,
    src window); each group split into dma_gathers of <=16 tiles
    (128 tokens each, 256B table rows). Gathers round-robin 4 SWDGE
    queues so all 4 Q7 core pairs generate descriptors concurrently
    (descriptor generation ~7ns/token is the core bottleneck).
    TensorEngine segment-reduces each tile into a feature-major PSUM
    accumulator via one-hot matrices M[token, dstoff], built on DVE in
    2x perf mode from an 8x-replicated dstoff array (all operands
    last-dim stride-1).
  - Gather windows are 32768 rows (int16 idx) at staggered bases so
    bucket overflow beyond 512 tokens sheds to the previous window,
    keeping schedule padding ~0.5%.
  - Self-loops via per-block direct loads of the bounce + identity-matmul
    accumulation.
  - Tails per 128-dst block, split in two stages (psum->SBUF copies at
    1-sbp lag, math at 2-sbp lag) so they never stall the gather/matmul
    streams: layer 1: z~ = dis*relu(dis*agg + b1) -> layer-2 bounce
    (partial AllGathers overlap the rest of layer 1); layer 2:
    h2 = agg @ W2, y = log_softmax(dis*h2 + b2) with Ln hoisted per sbp.
"""
import os
import sys

sys.path.insert(0, "/opt/trn_rl_repo")

import numpy as np

N, FIN, HD, C = 100000, 128, 64, 64
E = 1600000
NCORES = 8
S = 12544                  # slots per core (98 blocks of 128)
NT = S // 128              # 98 dst blocks
GROWS = NCORES * S         # 100352 global table rows
WINW = 32768               # gather window rows (int16 index limit)
BASES = [0, 22528, 45056, 67584]   # staggered window bases (overlapping)
NR = len(BASES)            # 4 windows; BASES[3]+WINW == GROWS
PAIRW = 16                 # dst blocks per superblock-pair (2 halves x 8)
NSBP = (NT + PAIRW - 1) // PAIRW          # 7 (16x6, 2)
HALF_BLKS = PAIRW // 2     # 8
HALF_COLS = HALF_BLKS * 128  # 1024 psum cols per half (2 banks)
MAXT = 16                  # max tiles (of 128 tokens) per gather
NQ = 4
# bounce/table pieces (2-sbp-aligned local row ranges). The global tables
# are laid out PIECE-MAJOR: [piece][core][rows] so each partial AllGather's
# output region is contiguous (collectives reject strided outputs). With
# 4096-row pieces, gather window w covers table pieces {w-1, w} exactly,
# so early windows unblock while later pieces still transfer. Fewer, bigger
# pieces also amortize the ~25us fixed cost per collective.
PBOUNDS = [0, 4096, 8192, 12288, 12544]    # sbp-aligned piece bounds
PIECES = list(zip(PBOUNDS[:-1], PBOUNDS[1:]))
PLEN = [r1 - r0 for r0, r1 in PIECES]
PBASE = [NCORES * r0 for r0, _ in PIECES]   # table row base of piece p
P_LAST_SBP = [(r1 - 1) // (PAIRW * 128) for _, r1 in PIECES]


def _grow_of(core, lr):
    """Global table row for (core, local row) under piece-major layout."""
    bounds = np.asarray(PBOUNDS, np.int64)
    p = np.searchsorted(bounds, lr, side="right") - 1
    r0 = bounds[p]
    plen = bounds[p + 1] - r0
    return NCORES * r0 + core * plen + (lr - r0)


def _assign_blocks(degs):
    """Round-based LPT: one node per block per round, heaviest node of the
    round to the lightest block. Keeps slot occupancy balanced (127/128)
    and block degree-sums within ~1 of each other.

    degs: per-node degrees, desc order. Returns block id per node.
    """
    M = len(degs)
    tot = np.zeros(NT, np.float64)
    out = np.empty(M, np.int64)
    i = 0
    while i < M:
        k = min(NT, M - i)
        ob = np.argsort(tot, kind="stable")[:k]
        out[i:i + k] = ob
        tot[ob] += degs[i:i + k]
        i += k
    return out


def _plan(x, edge_index):
    """Host-side planning. Returns per-core arrays + a uniform schedule."""
    x = np.asarray(x, np.float32)
    ei = np.asarray(edge_index, np.int64)
    src, dst = ei[0], ei[1]

    deg = np.bincount(dst, minlength=N).astype(np.float64) + 1.0  # + self loop
    dis_f = (1.0 / np.sqrt(deg)).astype(np.float32)

    # degree-balanced snake sharding across cores
    order = np.argsort(-deg, kind="stable")
    posn = np.arange(N)
    blk, rem = posn // NCORES, posn % NCORES
    corepat = np.where(blk % 2 == 0, rem, NCORES - 1 - rem).astype(np.int32)
    core_of = np.empty(N, np.int32)
    core_of[order] = corepat

    # block assignment per core; local row = block*128 + slot
    lrow_of = np.full(N, -1, np.int64)
    nodes_by_core = []
    for c in range(NCORES):
        nodes_c = order[core_of[order] == c]   # desc by degree
        bids = _assign_blocks(deg[nodes_c])
        slot = np.zeros(NT, np.int64)
        lr = np.empty(len(nodes_c), np.int64)
        for i, b in enumerate(bids):
            lr[i] = b * 128 + slot[b]
            slot[b] += 1
        lrow_of[nodes_c] = lr
        nodes_by_core.append(nodes_c)

    grow_of = _grow_of(core_of.astype(np.int64), lrow_of)

    # per-core token lists (real edges only; self-loops handled separately)
    # window assignment with downward rebalance: a token with src row in
    # [BASES[w], BASES[w-1]+WINW) can be served by window w-1 too; shed
    # bucket overflow beyond 512 tokens down a window (cap-aware).
    bases_a = np.asarray(BASES, np.int64)
    tok_src, tok_dst, tok_win = [], [], []
    dst_core = core_of[dst]
    for c in range(NCORES):
        m = dst_core == c
        ts = grow_of[src[m]]
        td = lrow_of[dst[m]]
        wn = np.searchsorted(bases_a[1:], ts, side="right")  # native window
        movable = (wn > 0) & (ts < bases_a[np.maximum(wn - 1, 0)] + WINW)
        b_of = td >> 7
        key = b_of * NR + wn
        osort = np.argsort(key, kind="stable")
        ts, td, wn, movable = ts[osort], td[osort], wn[osort], movable[osort]
        key_s = key[osort]
        starts = np.searchsorted(key_s, np.arange(NT * NR))
        ends = np.searchsorted(key_s, np.arange(NT * NR) + 1)
        wf = wn.copy()
        for b in range(NT):
            e = [int(ends[b * NR + w] - starts[b * NR + w])
                 for w in range(NR)]
            for w in (3, 2, 1):
                allow = e[w] - 512
                if w == 1:
                    allow = min(allow, 512 - e[0])
                if allow <= 0:
                    continue
                s0, s1 = starts[b * NR + w], ends[b * NR + w]
                mv = np.nonzero(movable[s0:s1])[0]
                mcnt = min(allow, len(mv))
                if mcnt > 0:
                    wf[s0 + mv[:mcnt]] = w - 1
                    e[w] -= mcnt
                    e[w - 1] += mcnt
        tok_src.append(ts)
        tok_dst.append(td)
        tok_win.append(wf)

    # counts per (core, block, window)
    cnt = np.zeros((NCORES, NT, NR), np.int64)
    for c in range(NCORES):
        b = tok_dst[c] >> 7
        np.add.at(cnt[c], (b, tok_win[c]), 1)

    ntok_br = cnt.max(axis=0)                    # [NT, NR]
    ntiles_br = -(-ntok_br // 128)               # ceil to 128-token tiles

    # schedule: per (sbp, r) group, tiles block-major, chunked into gathers
    # of <= MAXT tiles (balanced chunk sizes)
    gathers = []   # dicts: sbp, r, tiles(list of b), icol0, mm0
    icol = 0
    mmc = 0
    for sbp in range(NSBP):
        b0, b1 = sbp * PAIRW, min((sbp + 1) * PAIRW, NT)
        for r in range(NR):
            tiles = []
            for b in range(b0, b1):
                tiles += [b] * int(ntiles_br[b, r])
            if not tiles:
                continue
            k = -(-len(tiles) // MAXT)
            q, rem_ = divmod(len(tiles), k)
            pos = 0
            for j in range(k):
                sz = q + (1 if j < rem_ else 0)
                gathers.append({
                    "sbp": sbp, "r": r, "tiles": tiles[pos:pos + sz],
                    "icol0": icol, "mm0": mmc,
                })
                icol += sz * 8     # 128 tokens/tile / 16 = 8 idx cols
                mmc += sz
                pos += sz
    gcols = icol
    nmm_total = mmc

    # per-core gidx + dstoff arrays following the schedule
    def wrap16(v):
        n = len(v)
        a = np.asarray(v, np.int16).reshape(n // 16, 16).T.copy()
        return np.tile(a, (8, 1))

    import ml_dtypes
    gidx_all = np.zeros((NCORES, 128, gcols), np.int16)
    # dof8: each dstoff value replicated 8x along the innermost axis so the
    # on-chip M-build is_equal has stride-1 last dims on every operand
    # (DVE 2x perf mode; stride-0 innermost falls off the fast path).
    doff_all = np.full((NCORES, 128, nmm_total, 8), -1.0, ml_dtypes.bfloat16)

    for c in range(NCORES):
        b_of = tok_dst[c] >> 7
        key = b_of * NR + tok_win[c]
        osort = np.argsort(key, kind="stable")
        ts_s, td_s = tok_src[c][osort], tok_dst[c][osort]
        key_s = key[osort]
        starts = np.searchsorted(key_s, np.arange(NT * NR))
        ends = np.searchsorted(key_s, np.arange(NT * NR) + 1)
        cursor = np.array(starts)          # per-bucket consumption cursor

        for g in gathers:
            r = g["r"]
            ntg = len(g["tiles"])
            idx_loc = np.zeros(ntg * 128, np.int64)
            dof = np.full(ntg * 128, -1.0, np.float32)
            for j, b in enumerate(g["tiles"]):
                k = b * NR + r
                s0 = cursor[k]
                take = min(128, ends[k] - s0)
                if take > 0:
                    idx_loc[j * 128:j * 128 + take] = (
                        ts_s[s0:s0 + take] - BASES[r])
                    dof[j * 128:j * 128 + take] = (
                        td_s[s0:s0 + take] - b * 128).astype(np.float32)
                    cursor[k] += take
            gidx_all[c][:, g["icol0"]:g["icol0"] + ntg * 8] = wrap16(idx_loc)
            doff_all[c][:, g["mm0"]:g["mm0"] + ntg, :] = (
                dof.reshape(ntg, 128).T.astype(ml_dtypes.bfloat16)[:, :, None])

    # per-core xT (bf16, with dis pre-folded into rows so prep is a pure
    # matmul), dis with p = row%128, t = row//128
    xT_all = np.zeros((NCORES, 128, S), ml_dtypes.bfloat16)
    dis_all = np.ones((NCORES, 128, NT), np.float32)
    for c in range(NCORES):
        nodes_c = nodes_by_core[c]
        lr = lrow_of[nodes_c]
        xT_all[c][:, lr] = (
            x[nodes_c] * dis_f[nodes_c][:, None]).T.astype(ml_dtypes.bfloat16)
        dis_all[c][lr % 128, lr // 128] = dis_f[nodes_c]

    return {
        "gathers": gathers,
        "gcols": gcols,
        "nmm_total": nmm_total,
        "gidx": gidx_all,
        "doff": doff_all,
        "xT": xT_all,
        "dis": dis_all,
        "nodes_by_core": nodes_by_core,
        "lrow_of": lrow_of,
    }


def _build(plan, stage=99):
    import concourse.bacc as bacc
    import concourse.bass as bass
    import concourse.tile as tile
    import concourse.mybir as mybir

    f32 = mybir.dt.float32
    bf16 = mybir.dt.bfloat16
    i16 = mybir.dt.int16
    AF = mybir.ActivationFunctionType
    ALU = mybir.AluOpType

    gathers = plan["gathers"]
    gcols = plan["gcols"]
    nmm_total = plan["nmm_total"]
    max_icols = max(len(g["tiles"]) * 8 for g in gathers)

    nc = bacc.Bacc("TRN2", target_bir_lowering=False, debug=False,
                   num_devices=NCORES, num_swdge_queues=NQ)

    t_xT = nc.dram_tensor("xT", [128, S], bf16, kind="ExternalInput")
    t_dis = nc.dram_tensor("dis", [128, NT], f32, kind="ExternalInput")
    t_W1 = nc.dram_tensor("W1", [FIN, HD], bf16, kind="ExternalInput")
    t_W2b = nc.dram_tensor("W2b", [128, C], f32, kind="ExternalInput")
    t_b1 = nc.dram_tensor("b1b", [128, HD], f32, kind="ExternalInput")
    t_b2 = nc.dram_tensor("b2b", [128, C], f32, kind="ExternalInput")
    t_id2 = nc.dram_tensor("id2", [128, 64], f32, kind="ExternalInput")
    t_idf = nc.dram_tensor("idf", [128, 128], bf16, kind="ExternalInput")
    t_iob = nc.dram_tensor("iob", [128, 128], bf16, kind="ExternalInput")
    t_gi = nc.dram_tensor("gidx", [128, gcols], i16, kind="ExternalInput")
    t_do = nc.dram_tensor("doff", [128, nmm_total * 8], bf16,
                          kind="ExternalInput")
    t_cn = nc.dram_tensor("consts", [128, 640], f32, kind="ExternalInput")
    t_y = nc.dram_tensor("y", [S, C], f32, kind="ExternalOutput")
    t_warm = nc.dram_tensor("warm", [16, 16], f32, kind="Internal")
    t_warm_out = nc.dram_tensor("warm_out", [128, 16], f32, kind="Internal",
                                addr_space="Shared")

    # tables are bf16 padded to 128 cols (gather elem granularity is 256B).
    # bounces are split into pieces so partial AllGathers can fire as soon
    # as the producing phase completes each piece (overlap with compute).
    g1_bounce = [
        nc.dram_tensor(f"g1_bounce{i}", [r1 - r0, 128], bf16, kind="Internal")
        for i, (r0, r1) in enumerate(PIECES)]
    g2_bounce = [
        nc.dram_tensor(f"g2_bounce{i}", [r1 - r0, 128], bf16, kind="Internal")
        for i, (r0, r1) in enumerate(PIECES)]
    g1_table = nc.dram_tensor("g1_table", [GROWS, 128], bf16, kind="Internal",
                              addr_space="Shared")
    g2_table = nc.dram_tensor("g2_table", [GROWS, 128], bf16, kind="Internal",
                              addr_space="Shared")

    def bounce_ap(pieces, b):
        """AP for dst-block b's 128 rows within its bounce piece."""
        for t, (r0, r1) in zip(pieces, PIECES):
            if b * 128 >= r0 and (b + 1) * 128 <= r1:
                return t[b * 128 - r0:(b + 1) * 128 - r0, :]
        raise AssertionError(b)

    def ag_piece(table, pieces, i):
        """Partial AllGather of bounce piece i into its contiguous
        piece-major table region."""
        nc.gpsimd.collective_compute(
            "AllGather", mybir.AluOpType.bypass,
            replica_groups=[list(range(NCORES))],
            ins=[pieces[i][:]],
            outs=[table[PBASE[i]:PBASE[i] + NCORES * PLEN[i], :]])

    with tile.TileContext(nc) as tc:
        with tc.tile_pool(name="sbc", bufs=1) as sbc, \
             tc.tile_pool(name="sbx", bufs=2) as sbx, \
             tc.tile_pool(name="sbg", bufs=12) as sbg, \
             tc.tile_pool(name="sbm", bufs=8) as sbm, \
             tc.tile_pool(name="sbi", bufs=12) as sbi, \
             tc.tile_pool(name="sbt", bufs=3) as sbt, \
             tc.tile_pool(name="psa", bufs=3, space="PSUM") as psa, \
             tc.tile_pool(name="pst", bufs=2, space="PSUM") as pst:

            W1t = sbc.tile([FIN, HD], bf16)
            nc.sync.dma_start(out=W1t[:], in_=t_W1[:])
            W2bt = sbc.tile([128, C], f32)
            nc.sync.dma_start(out=W2bt[:], in_=t_W2b[:])
            b1t = sbc.tile([128, HD], f32)
            nc.sync.dma_start(out=b1t[:], in_=t_b1[:])
            b2t = sbc.tile([128, C], f32)
            nc.sync.dma_start(out=b2t[:], in_=t_b2[:])
            id2t = sbc.tile([128, 64], f32)
            nc.sync.dma_start(out=id2t[:], in_=t_id2[:])
            idft = sbc.tile([128, 128], bf16)
            nc.sync.dma_start(out=idft[:], in_=t_idf[:])
            iobt = sbc.tile([128, 128], bf16)
            nc.sync.dma_start(out=iobt[:], in_=t_iob[:])
            cons = sbc.tile([128, 640], f32)
            nc.sync.dma_start(out=cons[:], in_=t_cn[:])
            dist = sbc.tile([128, NT], f32)
            nc.sync.dma_start(out=dist[:], in_=t_dis[:])
            dof_sb = sbc.tile([128, nmm_total * 8], bf16)
            nc.sync.dma_start(out=dof_sb[:], in_=t_do[:])

            zrow = cons[0:1, 0:128]        # zeros [1, 128]
            zrhs = cons[0:1, 128:640]      # zeros [1, 512]

            # tiny dummy collective: pays the one-time collectives group
            # setup barrier (~40us) concurrently with prep instead of
            # delaying the first real AllGather piece
            if stage >= 2:
                nc.gpsimd.collective_compute(
                    "AllGather", mybir.AluOpType.bypass,
                    replica_groups=[list(range(NCORES))],
                    ins=[t_warm[:]], outs=[t_warm_out[:]])

            # ------- layer-1 prep: g1 = (dis*x) @ W1 (dis host-folded) ------
            CHB = 8                        # blocks per chunk (within a piece)
            g1_fired = 0
            for cb0 in range(0, NT, CHB):
                cb1 = min(cb0 + CHB, NT)
                nb = cb1 - cb0
                xc = sbx.tile([128, CHB * 128], bf16, tag="xc")
                nc.sync.dma_start(out=xc[:, 0:nb * 128],
                                  in_=t_xT[:, cb0 * 128:cb1 * 128])
                g1c = sbx.tile([128, CHB, 128], bf16, tag="g1c")
                for j in range(nb):
                    pp = pst.tile([128, 128], f32, tag="pp")
                    nc.tensor.matmul(out=pp[:, 0:HD],
                                     lhsT=xc[:, j * 128:(j + 1) * 128],
                                     rhs=W1t[:], start=True, stop=True)
                    nc.scalar.copy(g1c[:, j, 0:HD], pp[:, 0:HD])
                pi = next(i for i, (a, b_) in enumerate(PIECES)
                          if a <= cb0 * 128 < b_)  # chunk lies in one piece
                r0 = PIECES[pi][0]
                nc.sync.dma_start(
                    out=g1_bounce[pi][cb0 * 128 - r0:cb1 * 128 - r0, :]
                    .rearrange("(j p) f -> p j f", p=128),
                    in_=g1c[:, 0:nb, :])
                # fire partial AllGathers as soon as their rows are written
                if stage >= 2:
                    while (g1_fired < len(PIECES)
                           and PIECES[g1_fired][1] <= cb1 * 128):
                        ag_piece(g1_table, g1_bounce, g1_fired)
                        g1_fired += 1

            # ---------------- edge phase ----------------
            def edge_head(table, bounce, sbp, gq0):
                """Emit acc clears, self-loop matmuls, and gathers for sbp.
                Returns the psum acc tile."""
                b0 = sbp * PAIRW
                nblk = min(PAIRW, NT - b0)
                acc = psa.tile([128, HALF_COLS], f32, tag="acc")
                ncol_used = (min(nblk, HALF_BLKS)) * 128
                nseg = -(-ncol_used // 512)
                for seg in range(nseg):
                    nc.tensor.matmul(
                        out=acc[:, seg * 512:(seg + 1) * 512],
                        lhsT=zrow[:], rhs=zrhs[:],
                        start=True, stop=False, skip_group_check=True)
                # self-loop contribution: acc[block] += own_g.T via
                # accumulate-matmul with identity rhs
                for lb in range(nblk):
                    b = b0 + lb
                    half = lb // HALF_BLKS
                    col = (lb % HALF_BLKS) * 128
                    selfb = sbt.tile([128, 128], bf16, tag="selfb")
                    nc.sync.dma_start(
                        out=selfb[:], in_=bounce_ap(bounce, b))
                    nc.tensor.matmul(
                        out=acc[64 * half:64 * half + 64, col:col + 128],
                        lhsT=selfb[:, 0:HD], rhs=idft[:],
                        start=False, stop=False, skip_group_check=True,
                        tile_position=(0, 64 * half))
                # find last mm per 512-col psum seg to set stop
                sbp_gathers = [g for g in gathers if g["sbp"] == sbp]
                last_of_seg = {}
                for g in sbp_gathers:
                    for j, b in enumerate(g["tiles"]):
                        lb = b - b0
                        seg = ((lb % HALF_BLKS) * 128) // 512
                        last_of_seg[seg] = g["mm0"] + j
                gq = gq0
                for g in sbp_gathers:
                        r = g["r"]
                        tiles = g["tiles"]
                        ntg = len(tiles)
                        nidx = ntg * 128
                        r1 = min(BASES[r] + WINW, GROWS)
                        src_ap = table[BASES[r]:r1, :]
                        gi = sbi.tile([128, max_icols], i16, tag="gi")
                        nc.sync.dma_start(
                            out=gi[:, 0:ntg * 8],
                            in_=t_gi[:, g["icol0"]:g["icol0"] + ntg * 8])
                        buf = sbg.tile([128, MAXT, 128], bf16, tag="buf")
                        nc.gpsimd.dma_gather(
                            out_ap=buf[:, 0:ntg, :],
                            in_ap=src_ap,
                            idxs_ap=gi[:, 0:ntg * 8],
                            num_idxs=nidx,
                            num_idxs_reg=nidx,
                            elem_size=128,
                            queue_num=gq % NQ,
                            single_packet=False,
                        )
                        gq += 1
                        # M[p, j, d] = (dof[p, j] == d), built with every
                        # operand's last AP dim stride-1 (8-elem inner) so
                        # DVE runs in 2x perf mode:
                        #   out  [128, j, dh, dl] strides (128, 8, 1)
                        #   iota [128, j, dh, dl] strides (0, 8, 1)
                        #   dof8 [128, j, dh, dl] strides (8, 0, 1)
                        M = sbm.tile([128, MAXT, 128], bf16, tag="M")
                        do_sl = dof_sb[:, g["mm0"] * 8:(g["mm0"] + ntg) * 8]
                        nc.vector.tensor_tensor(
                            out=M[:, 0:ntg, :].rearrange(
                                "p j (dh dl) -> p j dh dl", dl=8),
                            in0=iobt[:].rearrange(
                                "p (dh dl) -> p dh dl", dl=8).unsqueeze(
                                1).broadcast_to([128, ntg, 16, 8]),
                            in1=do_sl.rearrange(
                                "p (j dl) -> p j dl", dl=8).unsqueeze(
                                2).broadcast_to([128, ntg, 16, 8]),
                            op=ALU.is_equal)
                        for j in range(ntg):
                            b = tiles[j]
                            lb = b - b0
                            half = lb // HALF_BLKS
                            col = (lb % HALF_BLKS) * 128
                            seg = col // 512
                            nc.tensor.matmul(
                                out=acc[64 * half:64 * half + 64,
                                        col:col + 128],
                                lhsT=buf[:, j, 0:HD],
                                rhs=M[:, j, :],
                                start=False,
                                stop=(g["mm0"] + j == last_of_seg.get(
                                    seg, -2)),
                                skip_group_check=True,
                                tile_position=(0, 64 * half))
                return acc, gq

            def edge_tails_a(acc, sbp):
                """Stage the sbp's psum accumulator to SBUF (frees psum
                early; runs on ACT)."""
                b0 = sbp * PAIRW
                nblk = min(PAIRW, NT - b0)
                pts = sbt.tile([128, PAIRW, 128], f32, tag="pts")
                for lb in range(nblk):
                    half = lb // HALF_BLKS
                    col = (lb % HALF_BLKS) * 128
                    hs, he = 64 * half, 64 * half + 64
                    nc.scalar.copy(pts[hs:he, lb, :],
                                   acc[hs:he, col:col + 128])
                return pts

            def edge_tails_b(pts, sbp, layer):
                b0 = sbp * PAIRW
                nblk = min(PAIRW, NT - b0)
                if layer == 2:
                    lgk = sbt.tile([128, PAIRW, C], f32, tag="lgk")
                    nmax24 = sbt.tile([128, PAIRW], f32, tag="nmax24")
                    sume24 = sbt.tile([128, PAIRW], f32, tag="sume24")
                for lb in range(nblk):
                    b = b0 + lb
                    half = lb // HALF_BLKS
                    hs, he = 64 * half, 64 * half + 64
                    if layer == 1:
                        tr = pst.tile([128, 128], f32, tag="pp")
                        nc.tensor.transpose(out=tr[:, 0:64],
                                            in_=pts[hs:he, lb, :],
                                            identity=id2t[hs:he, :])
                        t1m = sbt.tile([128, HD], f32, tag="t1m")
                        nc.vector.tensor_tensor(
                            out=t1m[:], in0=tr[:, 0:64],
                            in1=dist[:, b:b + 1].broadcast_to([128, HD]),
                            op=ALU.mult)
                        t1v = sbt.tile([128, HD], f32, tag="t1v")
                        nc.vector.tensor_tensor(
                            out=t1v[:], in0=t1m[:], in1=b1t[:],
                            op=ALU.add)
                        zt = sbt.tile([128, HD], f32, tag="zt")
                        nc.scalar.activation(zt[:], t1v[:], AF.Relu)
                        zs = sbt.tile([128, 2, HD], bf16, tag="zs")
                        # write z~ into both 64-col halves so the L2
                        # transposed self-load is valid on either
                        # partition half
                        nc.vector.tensor_tensor(
                            out=zs[:],
                            in0=zt[:].unsqueeze(1).broadcast_to(
                                [128, 2, HD]),
                            in1=dist[:, b:b + 1].unsqueeze(2).broadcast_to(
                                [128, 2, HD]),
                            op=ALU.mult)
                        nc.scalar.dma_start(
                            out=bounce_ap(g2_bounce, b),
                            in_=zs[:])
                    else:
                        h2T = pst.tile([128, 128], f32, tag="pp")
                        nc.tensor.matmul(out=h2T[0:64, :],
                                         lhsT=W2bt[hs:he, :],
                                         rhs=pts[hs:he, lb, :],
                                         start=True, stop=True,
                                         tile_position=(64 * half, 0))
                        h2Ts = sbt.tile([128, 128], f32, tag="h2Ts")
                        nc.scalar.copy(h2Ts[0:64, :], h2T[0:64, :])
                        h2 = pst.tile([128, 128], f32, tag="pp")
                        nc.tensor.transpose(out=h2[:, 0:64],
                                            in_=h2Ts[0:64, :],
                                            identity=id2t[0:64, :])
                        lgm = sbt.tile([128, C], f32, tag="lgm")
                        nc.vector.tensor_tensor(
                            out=lgm[:], in0=h2[:, 0:64],
                            in1=dist[:, b:b + 1].broadcast_to([128, C]),
                            op=ALU.mult)
                        nc.vector.tensor_tensor(
                            out=lgk[:, lb, :], in0=lgm[:], in1=b2t[:],
                            op=ALU.add)
                        nc.vector.tensor_reduce(
                            out=nmax24[:, lb:lb + 1], in_=lgk[:, lb, :],
                            axis=mybir.AxisListType.X,
                            op=ALU.max, negate=True)
                        ex = sbt.tile([128, C], f32, tag="ex")
                        nc.scalar.activation(
                            ex[:], lgk[:, lb, :], AF.Exp,
                            bias=nmax24[:, lb:lb + 1], scale=1.0,
                            accum_out=sume24[:, lb:lb + 1])
                if layer == 2:
                    # hoisted: one Ln per sbp (avoids per-block Exp/Ln
                    # activation-table reload thrash on the Scalar engine)
                    lse24 = sbt.tile([128, PAIRW], f32, tag="lse24")
                    nc.scalar.activation(lse24[:, 0:nblk],
                                         sume24[:, 0:nblk], AF.Ln)
                    cc24 = sbt.tile([128, PAIRW], f32, tag="cc24")
                    nc.vector.tensor_tensor(
                        out=cc24[:, 0:nblk], in0=nmax24[:, 0:nblk],
                        in1=lse24[:, 0:nblk], op=ALU.subtract)
                    for lb in range(nblk):
                        b = b0 + lb
                        yt = sbt.tile([128, C], f32, tag="yt")
                        nc.vector.tensor_tensor(
                            out=yt[:], in0=lgk[:, lb, :],
                            in1=cc24[:, lb:lb + 1].broadcast_to([128, C]),
                            op=ALU.add)
                        nc.scalar.dma_start(
                            out=t_y[b * 128:(b + 1) * 128, :], in_=yt[:])

            def edge_phase(table, bounce, layer):
                # tails_a (psum->SBUF stage) at 1-sbp lag frees the psum
                # accumulator; tails_b (transposes + math + writes) at 2-sbp
                # lag so its ops never stall the gather/M/matmul streams.
                # layer-1 fires partial g2 AllGathers as tails_b complete.
                gq = 0
                pend_a = None
                pend_b = None
                g2_fired = 0

                def fire_ags(done_sbp):
                    nonlocal g2_fired
                    if layer == 1 and stage >= 4:
                        while (g2_fired < len(PIECES)
                               and P_LAST_SBP[g2_fired] <= done_sbp):
                            ag_piece(g2_table, g2_bounce, g2_fired)
                            g2_fired += 1

                for sbp in range(NSBP):
                    acc, gq = edge_head(table, bounce, sbp, gq)
                    if pend_a is not None:
                        pts = edge_tails_a(pend_a[0], pend_a[1])
                        if pend_b is not None:
                            edge_tails_b(pend_b[0], pend_b[1], layer)
                            fire_ags(pend_b[1])
                        pend_b = (pts, pend_a[1])
                    pend_a = (acc, sbp)
                pts = edge_tails_a(pend_a[0], pend_a[1])
                if pend_b is not None:
                    edge_tails_b(pend_b[0], pend_b[1], layer)
                    fire_ags(pend_b[1])
                edge_tails_b(pts, pend_a[1], layer)
                fire_ags(pend_a[1])

            if stage >= 3:
                edge_phase(g1_table, g1_bounce, 1)

            if stage >= 5:
                edge_phase(g2_table, g2_bounce, 2)

    nc.compile()
    return nc


def _run(inputs, trace=False):
    import concourse.bass_utils as bass_utils

    x = np.asarray(inputs["x"], np.float32)
    W1 = np.asarray(inputs["W1"], np.float32)
    b1 = np.asarray(inputs["b1"], np.float32)
    W2 = np.asarray(inputs["W2"], np.float32)
    b2 = np.asarray(inputs["b2"], np.float32)

    plan = _plan(x, inputs["edge_index"])
    nc = _build(plan, stage=int(os.environ.get("KSTAGE", "99")))

    import ml_dtypes
    b1b = np.tile(b1[None, :], (128, 1)).astype(np.float32)
    b2b = np.tile(b2[None, :], (128, 1)).astype(np.float32)
    W2b = np.tile(W2, (2, 1)).astype(np.float32)
    id2 = np.tile(np.eye(64, dtype=np.float32), (2, 1))
    idf = np.eye(128, dtype=ml_dtypes.bfloat16)
    iob = np.tile(np.arange(128, dtype=ml_dtypes.bfloat16)[None, :], (128, 1))
    consts = np.zeros((128, 640), np.float32)

    in_maps = []
    for c in range(NCORES):
        in_maps.append({
            "xT": plan["xT"][c],
            "dis": plan["dis"][c],
            "W1": W1.astype(ml_dtypes.bfloat16),
            "W2b": W2b, "b1b": b1b, "b2b": b2b,
            "id2": id2, "idf": idf, "iob": iob,
            "gidx": plan["gidx"][c],
            "doff": plan["doff"][c].reshape(128, -1),
            "consts": consts,
        })

    res = bass_utils.run_bass_kernel_spmd(
        nc, in_maps, core_ids=list(range(NCORES)), trace=trace)

    out = np.empty((N, C), np.float32)
    lrow_of = plan["lrow_of"]
    for c in range(NCORES):
        yc = np.asarray(res.results[c]["y"], np.float32)
        nodes_c = plan["nodes_by_core"][c]
        out[nodes_c] = yc[lrow_of[nodes_c]]
    return out, res


def kernel(**inputs):
    out, _ = _run(inputs, trace=False)
    return out
